# revision 1
# baseline (speedup 1.0000x reference)
"""BEV voxel-pooling kernel for 8 TRN2 NeuronCores (data-parallel over batch).

Design (constraints established by on-HW probes):
  * dma_scatter_add loses duplicate indices WITHIN one instruction (every
    descriptor RMWs the pre-instruction value, last writer wins), but
    accumulates exactly ACROSS serialized instructions. Descriptor ring caps
    one instruction at ~1024 descriptors.
  => tokens are scattered in 1024-token windows; the host guarantees each
     window's live indices are unique (duplicate occurrences are diverted to a
     small overflow stream, re-windowed with the same property). Zero-payload
     tokens (out-of-grid, padding, diverted) are pointed at a slot that no
     live token of the same window uses, so their old+0 writes are harmless.

  Per core (1 batch): grid[32768 pair-slots, 128] f32; pair-slot =
  gx*128 + gy//2; a token's 64 features are placed in the low/high half of a
  512B payload by gy parity. Device: loads x (64MB), builds masked payloads
  on the DVE, runs the serialized scatter-add chain. Host: voxelization
  (exact f32 mirror of the reference math), window-dedup, final layout
  transpose of the device-produced voxel-major grid.
"""
import numpy as np

import concourse.bacc as bacc
import concourse.tile as tile
from concourse import mybir
from concourse.bass_utils import run_bass_kernel_spmd

F32 = mybir.dt.float32
I16 = mybir.dt.int16

B, N, D, H, W, C = 8, 6, 59, 16, 44, 64
NX, NY = 256, 256
NP = N * D * H * W            # 249216 points per batch/core
WIN = 1024                    # tokens per scatter instruction
NWIN = (NP + WIN - 1) // WIN  # 244
TOKPAD = NWIN * WIN           # 249856
COLS = TOKPAD // 128          # 1952
OF_BIG = 48                   # overflow: 48 full 1024-token windows (dense)
OF_SMALL = 256                # + 256 small 128-token windows (hot-slot tails)
OF_TOK = OF_BIG * WIN + OF_SMALL * 128   # 81920
OF_COLS = OF_TOK // 128       # 640
NSLOT = NX * (NY // 2)        # 32768 pair slots

_nc_cache = None


def _build():
    nc = bacc.Bacc("TRN2", target_bir_lowering=False, debug=False)
    xw = nc.dram_tensor("xw", [128, COLS, 64], F32, kind="ExternalInput")
    mpw = nc.dram_tensor("mpw", [128, COLS, 2], F32, kind="ExternalInput")
    idxw = nc.dram_tensor("idxw", [128, TOKPAD // 16], I16, kind="ExternalInput")
    ofx = nc.dram_tensor("ofx", [128, OF_COLS, 64], F32, kind="ExternalInput")
    ofmp = nc.dram_tensor("ofmp", [128, OF_COLS, 2], F32, kind="ExternalInput")
    ofidx = nc.dram_tensor("ofidx", [128, OF_TOK // 16], I16, kind="ExternalInput")
    grids = [
        nc.dram_tensor(f"grid{k}", [NSLOT, 128], F32, kind="ExternalOutput")
        for k in range(2)
    ]

    with tile.TileContext(nc) as tc:
        with tc.tile_pool(name="p", bufs=4) as pool:
            wcount = [0]

            def do_chunk(xsrc, mpsrc, idxsrc, col0, w):
                x_t = pool.tile([128, w, 64], F32, tag="x")
                mp_t = pool.tile([128, w, 2], F32, tag="mp")
                idx_t = pool.tile([128, w * 8], I16, tag="ix")
                pk_t = pool.tile([128, w, 128], F32, tag="pk")
                nc.sync.dma_start(out=x_t[:], in_=xsrc.ap()[:, col0:col0 + w, :])
                nc.sync.dma_start(out=mp_t[:], in_=mpsrc.ap()[:, col0:col0 + w, :])
                nc.sync.dma_start(
                    out=idx_t[:], in_=idxsrc.ap()[:, col0 * 8:(col0 + w) * 8]
                )
                for u in (0, 1):
                    nc.vector.tensor_tensor(
                        out=pk_t[:, :, 64 * u:64 * u + 64],
                        in0=x_t[:],
                        in1=mp_t[:, :, u:u + 1].broadcast_to([128, w, 64]),
                        op=mybir.AluOpType.mult,
                    )
                for j in range(w // 8):
                    g = grids[wcount[0] & 1]
                    wcount[0] += 1
                    nc.gpsimd.dma_scatter_add(
                        g.ap()[:],
                        pk_t[:, 8 * j:8 * j + 8, :],
                        idx_t[:, 64 * j:64 * j + 64],
                        WIN,
                        WIN,
                        128,
                    )

            def do_chunk_small(col0, w):
                """w cols, each col = one 128-token window."""
                x_t = pool.tile([128, w, 64], F32, tag="x")
                mp_t = pool.tile([128, w, 2], F32, tag="mp")
                idx_t = pool.tile([128, w * 8], I16, tag="ix")
                pk_t = pool.tile([128, w, 128], F32, tag="pk")
                nc.sync.dma_start(out=x_t[:], in_=ofx.ap()[:, col0:col0 + w, :])
                nc.sync.dma_start(out=mp_t[:], in_=ofmp.ap()[:, col0:col0 + w, :])
                nc.sync.dma_start(
                    out=idx_t[:], in_=ofidx.ap()[:, col0 * 8:(col0 + w) * 8]
                )
                for u in (0, 1):
                    nc.vector.tensor_tensor(
                        out=pk_t[:, :, 64 * u:64 * u + 64],
                        in0=x_t[:],
                        in1=mp_t[:, :, u:u + 1].broadcast_to([128, w, 64]),
                        op=mybir.AluOpType.mult,
                    )
                for j in range(w):
                    g = grids[wcount[0] & 1]
                    wcount[0] += 1
                    nc.gpsimd.dma_scatter_add(
                        g.ap()[:],
                        pk_t[:, j:j + 1, :],
                        idx_t[:, 8 * j:8 * j + 8],
                        128,
                        128,
                        128,
                    )

            # 30 chunks x 64 cols (8 windows) + 1 chunk x 32 cols (4 windows)
            for ci in range(30):
                do_chunk(xw, mpw, idxw, ci * 64, 64)
            do_chunk(xw, mpw, idxw, 30 * 64, 32)
            # overflow big windows: 48 windows = 6 chunks x 64 cols
            for ci in range(6):
                do_chunk(ofx, ofmp, ofidx, ci * 64, 64)
            # overflow small windows: 256 cols, 4 chunks x 64
            for ci in range(4):
                do_chunk_small(OF_BIG * 8 + ci * 64, 64)

    nc.compile()
    return nc


def _wrap16(tok):
    """token stream [T] -> [128, T//16] int16 (replicated across 8 groups)."""
    t16 = tok.reshape(-1, 16).T.astype(np.int16)      # [16, T//16]
    return np.tile(t16, (8, 1))


def _free_slot(used, w):
    s = (w * 977 + 13) % NSLOT
    while s in used:
        s = (s + 1) % NSLOT
    return s


def _prep_core(xb, slot, kept, par):
    """Build per-core device inputs. xb [NP,64] f32; slot/kept/par [NP]."""
    slot_p = np.zeros(TOKPAD, np.int64)
    kept_p = np.zeros(TOKPAD, bool)
    par_p = np.zeros(TOKPAD, np.int64)
    slot_p[:NP] = slot
    kept_p[:NP] = kept
    par_p[:NP] = par

    idx_main = np.zeros(TOKPAD, np.int64)
    m_main = np.zeros((TOKPAD, 2), np.float32)
    of_list = []   # (point_id, slot, parity)
    for w in range(NWIN):
        lo, hi = w * WIN, (w + 1) * WIN
        sl = slot_p[lo:hi]
        kp = kept_p[lo:hi]
        live = np.nonzero(kp)[0]
        _, first_pos = np.unique(sl[live], return_index=True)
        keepers = live[first_pos]
        dups = np.setdiff1d(live, keepers, assume_unique=False)
        used = set(sl[keepers].tolist())
        dead = _free_slot(used, w)
        idx_main[lo:hi] = dead
        idx_main[lo + keepers] = sl[keepers]
        m_main[lo + keepers, par_p[lo + keepers]] = 1.0
        for d in dups:
            of_list.append((lo + d, sl[d], par_p[lo + d]))

    # overflow: greedy re-window; window w holds at most one occurrence of a
    # slot. Mixed capacities: OF_BIG full windows then OF_SMALL 128-wide.
    n_w = OF_BIG + OF_SMALL
    caps = [WIN] * OF_BIG + [128] * OF_SMALL
    bases = np.concatenate([[0], np.cumsum(caps)])[:-1]
    of_windows = [[] for _ in range(n_w)]
    nxt = {}
    for rec in of_list:
        w = nxt.get(rec[1], 0)
        while w < n_w and len(of_windows[w]) >= caps[w]:
            w += 1
        assert w < n_w, f"overflow capacity exceeded ({len(of_list)} records)"
        of_windows[w].append(rec)
        nxt[rec[1]] = w + 1

    of_idx = np.zeros(OF_TOK, np.int64)
    of_m = np.zeros((OF_TOK, 2), np.float32)
    of_x = np.zeros((OF_TOK, 64), np.float32)
    for wi in range(n_w):
        recs = of_windows[wi]
        used = {r[1] for r in recs}
        dead = _free_slot(used, 10_000 + wi)
        b0 = bases[wi]
        of_idx[b0:b0 + caps[wi]] = dead
        for k, (pid, s, p_) in enumerate(recs):
            of_idx[b0 + k] = s
            of_m[b0 + k, p_] = 1.0
            of_x[b0 + k] = xb[pid]

    xpad = np.zeros((TOKPAD, 64), np.float32)
    xpad[:NP] = xb
    return {
        "xw": np.ascontiguousarray(
            xpad.reshape(COLS, 128, 64).transpose(1, 0, 2)),
        "mpw": np.ascontiguousarray(
            m_main.reshape(COLS, 128, 2).transpose(1, 0, 2)),
        "idxw": _wrap16(idx_main),
        "ofx": np.ascontiguousarray(
            of_x.reshape(OF_COLS, 128, 64).transpose(1, 0, 2)),
        "ofmp": np.ascontiguousarray(
            of_m.reshape(OF_COLS, 128, 2).transpose(1, 0, 2)),
        "ofidx": _wrap16(of_idx),
    }


def kernel(x, geom, dx, bx):
    global _nc_cache
    x = np.asarray(x, np.float32)
    geom = np.asarray(geom, np.float32)
    dx = np.asarray(dx, np.float32)
    bx = np.asarray(bx, np.float32)

    # exact f32 mirror of the reference voxelization
    off = (bx - dx / np.float32(2.0)).astype(np.float32)
    g = ((geom - off) / dx).astype(np.int32)       # trunc toward zero
    g = g.reshape(B, NP, 3)
    kept = ((g[..., 0] >= 0) & (g[..., 0] < NX)
            & (g[..., 1] >= 0) & (g[..., 1] < NY)
            & (g[..., 2] >= 0) & (g[..., 2] < 1))
    gx = g[..., 0].astype(np.int64)
    gy = g[..., 1].astype(np.int64)
    slot = np.where(kept, gx * 128 + gy // 2, 0)
    par = np.where(kept, gy & 1, 0)

    xf = x.reshape(B, NP, 64)
    in_maps = [
        _prep_core(xf[b], slot[b], kept[b], par[b]) for b in range(B)
    ]

    if _nc_cache is None:
        _nc_cache = _build()
    import time as _time
    _t0 = _time.perf_counter()
    res = run_bass_kernel_spmd(_nc_cache, in_maps, core_ids=list(range(8)))
    global LAST_DEVICE_CALL_S
    LAST_DEVICE_CALL_S = _time.perf_counter() - _t0

    out = np.empty((B, 64, NX, NY), np.float32)
    for b in range(B):
        gr = res.results[b]["grid0"] + res.results[b]["grid1"]   # [32768, 128]
        gr = gr.reshape(NX, NY // 2, 2, 64)         # gx, gy//2, gy&1, c
        out[b] = gr.transpose(3, 0, 1, 2).reshape(64, NX, NY)
    return out

